# revision 23
# baseline (speedup 1.0000x reference)
"""Trainium2 Bass kernel for nn_ChemicalDevelopment (drag-scan + separable
Gaussian blur + mask-combine + 3x3 channel coupling + tanh saturation).

Self-contained: hardcodes shapes/sharding. Shards the W (column) axis across
8 NeuronCores (512 cols each, 1-col blur halo); each core processes its
full-height column slab independently (no collectives).

v5: weight matrices are zero-padded to M=128 columns with out-row r mapped
to partition r+1, so matmul outputs are partition-aligned with the input
tile rows (no shifted xs3 copy needed) and FWL (fast weight load) engages.
Channel mix j=0 stays on PE (seed + 3 diag matmuls into PSUM); j=1,2 move
to DVE scalar_tensor_tensor chains. Elementwise spread: scalar engine does
2 PSUM scatter-copies + tanh, gpsimd does 1 copy + u3, DVE does d3/p3/mix.
"""
import numpy as np

H_FULL = 4096
W_FULL = 4096
NCORES = 8
WS = W_FULL // NCORES      # 512 columns per core
RH = 1                     # truncated blur radius (taps |e|>1 ~5e-4 mass)
WP = WS + 2 * RH           # padded plane width (514)
P = 128                    # partition block (rows)
OUT_R = P - 2 * RH         # 126 output rows per tile
NB = -(-H_FULL // OUT_R)   # 33 tiles
PAD_T = RH                 # one zero row above the image
H_PAD = OUT_R * (NB - 1) + P  # 4160 padded rows
PAD_B = H_PAD - H_FULL - PAD_T
HIST = 62                  # scan history rows from previous tile
SIGMA_SOFT = 2.0
SIGMA_HARD = 0.5
D_MAX = 3.0
SINV = 1.0 / (D_MAX + 1e-6)
DMX = D_MAX + 1e-6         # baked into T/U/B to unscale xq
F = 3 * WP                 # SBUF x-tile free width (3*514=1542)
FC = 3 * WS                # output width (1536)

_NC_CACHE = {}


def _taps():
    # identical arithmetic to the reference (f32), truncated to radius RH
    # and renormalized
    x = np.arange(-12, 13, dtype=np.float32)
    k = np.exp(np.float32(-0.5) * (x / np.float32(SIGMA_HARD)) ** 2)
    k = k / k.sum()
    kept = k[12 - RH:12 + RH + 1].astype(np.float64)
    return kept / kept.sum()


def _matrices():
    # All (128, 128) fp16, out row r at column m=r+1 (cols 0 and 127 zero)
    d = np.exp(-1.0 / SIGMA_SOFT)
    scale = (1.0 - d) * DMX
    k = np.arange(P)[:, None]          # in-tile partition (row)
    m = np.arange(P)[None, :]          # out partition (row r = m-1)
    valid = (m >= 1) & (m <= OUT_R)
    e = m - k
    with np.errstate(under="ignore"):
        T = np.where(valid & (e >= 0), scale * d ** np.clip(e, 0, None), 0.0)
        # history: partitions 64+h (h=0..61) hold prev-tile rows
        U = np.zeros((P, P))
        h = np.arange(HIST)[:, None]
        U[64:64 + HIST, :] = np.where(
            valid, scale * d ** ((m - 1) + 63 - h), 0.0)
    kt = _taps()
    B = []
    for i in range(2 * RH + 1):
        band = np.where(valid & (np.abs(k - m) <= RH),
                        kt[np.clip(k - m + RH, 0, 2 * RH)], 0.0)
        B.append(kt[i] * band * DMX)
    f = lambda a: np.ascontiguousarray(a, np.float16)
    return f(T), f(U), [f(b) for b in B]


def _build_nc(nb, last_rows):
    import concourse.bacc as bacc
    import concourse.mybir as mybir
    from concourse.tile import TileContext

    f32 = mybir.dt.float32
    f16 = mybir.dt.float16
    AO = mybir.AluOpType

    T, U, B = _matrices()
    NW = 2 + len(B)                  # weight blocks: T, U, B*3
    wconst_np = np.zeros((P, NW * P), np.float16)
    wconst_np[:, 0:P] = T
    wconst_np[:, P:2 * P] = U
    for i, b in enumerate(B):
        wconst_np[:, (2 + i) * P:(3 + i) * P] = b
    ident_np = np.ascontiguousarray(np.eye(P, dtype=np.float16))

    nc = bacc.Bacc(trn_type="TRN2", debug=False)
    hx = nc.dram_tensor("x", [H_PAD, F], f16, kind="ExternalInput")
    hcm = nc.dram_tensor("cmat", [P, 9], f32, kind="ExternalInput")
    hy = nc.dram_tensor("y", [H_FULL, FC], f16, kind="ExternalOutput")
    hconst = nc.inline_tensor(wconst_np, name="wconst")
    hident = nc.inline_tensor(ident_np, name="ident")

    with TileContext(nc) as tc:
        with tc.tile_pool(name="wpool", bufs=1) as wpool, \
             tc.tile_pool(name="xpool", bufs=4) as xpool, \
             tc.tile_pool(name="spool", bufs=3) as spool, \
             tc.tile_pool(name="upool", bufs=2) as upool, \
             tc.tile_pool(name="opool", bufs=2) as opool, \
             tc.tile_pool(name="pshpool", bufs=3, space="PSUM") as pshpool, \
             tc.tile_pool(name="psvpool", bufs=1, space="PSUM") as psvpool:

            wconst = wpool.tile([P, NW * P], f16, name="wconst_t")
            nc.sync.dma_start(out=wconst, in_=hconst[:, :])
            wT = wconst[:, 0:P]
            wU = wconst[64:64 + HIST, P:2 * P]
            wB = [wconst[:, (2 + i) * P:(3 + i) * P] for i in range(len(B))]
            ident = wpool.tile([P, P], f16, name="ident_t")
            nc.sync.dma_start(out=ident, in_=hident[:, :])

            # negc[p, 3i+j] = -C[i,j]*SINV on every partition (host bcast)
            cmsb = wpool.tile([P, 9], f32, name="cmsb")
            nc.sync.dma_start(out=cmsb, in_=hcm[:, :])
            negc = wpool.tile([P, 9], f32, name="negc")
            nc.scalar.mul(negc, cmsb, -SINV)
            # diag mix-weight tiles diag(-C[i,j]*SINV), indexed 3*i+j
            wmix = []
            for kk in range(9):
                dg = wpool.tile([P, P], f16, name=f"wmix{kk}")
                nc.vector.tensor_scalar_mul(out=dg, in0=ident,
                                            scalar1=negc[:, kk:kk + 1])
                wmix.append(dg)

            x_tiles = [None] * nb

            def load(b):
                xt = xpool.tile([P, F], f16, name=f"x{b}", tag="x")
                nc.sync.dma_start(out=xt, in_=hx[b * OUT_R:b * OUT_R + P, :])
                x_tiles[b] = xt

            def process(b):
                xb = x_tiles[b]
                xp = x_tiles[b - 1] if b > 0 else None

                s3 = spool.tile([P, FC], f16, name=f"s3_{b}", tag="s3")
                d3 = spool.tile([P, FC], f16, name=f"d3_{b}", tag="d3")

                for p in range(3):
                    c0 = p * WP
                    ctr = slice(c0 + RH, c0 + RH + WS)
                    ps_sh = pshpool.tile([P, 2 * WS], f32,
                                         name=f"ps_sh{b}_{p}", tag="ps_sh")
                    nc.tensor.matmul(out=ps_sh[:, 0:WS], lhsT=wT,
                                     rhs=xb[:, ctr],
                                     start=True, stop=(xp is None))
                    if xp is not None:
                        nc.tensor.matmul(out=ps_sh[:, 0:WS], lhsT=wU,
                                         rhs=xp[64:64 + HIST, ctr],
                                         start=False, stop=True,
                                         tile_position=(64, 0))
                    for i in range(len(B)):
                        dd = i - RH
                        sl = slice(c0 + RH + dd, c0 + RH + dd + WS)
                        nc.tensor.matmul(out=ps_sh[:, WS:2 * WS],
                                         lhsT=wB[i], rhs=xb[:, sl],
                                         start=(i == 0),
                                         stop=(i == len(B) - 1))
                    # copy s to SBUF (f32 -> f16); d = h(PSUM) - s(SBUF)
                    s3p = s3[:, p * WS:(p + 1) * WS]
                    if p == 2:
                        nc.vector.tensor_copy(out=s3p, in_=ps_sh[:, 0:WS])
                    else:
                        nc.scalar.copy(out=s3p, in_=ps_sh[:, 0:WS])
                    nc.vector.tensor_sub(out=d3[:, p * WS:(p + 1) * WS],
                                         in0=ps_sh[:, WS:2 * WS], in1=s3p)

                # p3 = xq (at out-row partitions) * d3
                p3 = spool.tile([P, FC], f16, name=f"p3_{b}", tag="p3")
                nc.vector.tensor_mul(
                    out=p3.rearrange("p (g w) -> p g w", g=3),
                    in0=xb.rearrange("p (g w) -> p g w", g=3)[:, :, RH:RH + WS],
                    in1=d3.rearrange("p (g w) -> p g w", g=3))
                # per-plane u3: the mix matmuls for plane i start as soon
                # as u3_i lands instead of after one 3.1us full-width op
                u3 = upool.tile([P, FC], f16, name=f"u3_{b}", tag="u3")
                for p in range(3):
                    sl = slice(p * WS, (p + 1) * WS)
                    nc.gpsimd.tensor_add(out=u3[:, sl], in0=s3[:, sl],
                                         in1=p3[:, sl])

                # channel mix j=0,1 on PE: seed xq_j, accumulate -cs_ij*u3_i
                ps_v = psvpool.tile([P, 2 * WS], f32, name=f"ps_v{b}",
                                    tag="ps_v")
                for j in (0, 1):
                    nc.tensor.matmul(
                        out=ps_v[:, j * WS:(j + 1) * WS], lhsT=ident,
                        rhs=xb[:, j * WP + RH:j * WP + RH + WS],
                        start=True, stop=False, skip_group_check=True)
                for i in range(3):
                    for j in (0, 1):
                        nc.tensor.matmul(out=ps_v[:, j * WS:(j + 1) * WS],
                                         lhsT=wmix[3 * i + j],
                                         rhs=u3[:, i * WS:(i + 1) * WS],
                                         start=False,
                                         stop=(i == 2), skip_group_check=True)

                # channel mix j=2 on DVE: v_2 = xq_2 - sum_i cs_i2*u3_i
                v3 = spool.tile([P, WS], f16, name=f"v3_{b}", tag="v3")
                nc.vector.scalar_tensor_tensor(
                    out=v3, in0=u3[:, 0:WS], scalar=negc[:, 2:3],
                    in1=xb[:, 2 * WP + RH:2 * WP + RH + WS],
                    op0=AO.mult, op1=AO.add)
                for i in (1, 2):
                    nc.vector.scalar_tensor_tensor(
                        out=v3, in0=u3[:, i * WS:(i + 1) * WS],
                        scalar=negc[:, 3 * i + 2:3 * i + 3],
                        in1=v3, op0=AO.mult, op1=AO.add)

                ot = opool.tile([P, FC], f16, name=f"o{b}", tag="o")
                nc.scalar.activation(out=ot[:, 0:2 * WS], in_=ps_v,
                                     func=mybir.ActivationFunctionType.Tanh)
                nc.scalar.activation(out=ot[:, 2 * WS:FC], in_=v3,
                                     func=mybir.ActivationFunctionType.Tanh)

                rows = last_rows if b == nb - 1 else OUT_R
                nc.sync.dma_start(out=hy[b * OUT_R:b * OUT_R + rows, :],
                                  in_=ot[RH:RH + rows, :])

            load(0)
            if nb > 1:
                load(1)
            for b in range(nb):
                if b + 2 < nb:
                    load(b + 2)
                process(b)

    nc.finalize()
    return nc


def _get_nc():
    key = (NB, H_FULL - OUT_R * (NB - 1))
    if key not in _NC_CACHE:
        _NC_CACHE[key] = _build_nc(NB, H_FULL - OUT_R * (NB - 1))
    return _NC_CACHE[key]


def make_in_maps(D_macro, coupling_matrix):
    D = np.asarray(D_macro, dtype=np.float32)
    C = np.asarray(coupling_matrix, np.float32).reshape(1, 9)
    Cb = np.ascontiguousarray(np.broadcast_to(C, (P, 9)))
    Dp = np.pad(D * np.float32(SINV),
                ((PAD_T, PAD_B), (RH, RH), (0, 0))).astype(np.float16)
    DT = np.ascontiguousarray(Dp.transpose(0, 2, 1))  # (H_PAD, 3, W+2RH)
    in_maps = []
    for m in range(NCORES):
        sl = np.ascontiguousarray(
            DT[:, :, m * WS:m * WS + WP]).reshape(H_PAD, F)
        in_maps.append({"x": sl, "cmat": Cb})
    return in_maps


def kernel(D_macro, coupling_matrix):
    from concourse.bass_utils import run_bass_kernel_spmd

    in_maps = make_in_maps(D_macro, coupling_matrix)
    nc = _get_nc()
    res = run_bass_kernel_spmd(nc, in_maps, core_ids=list(range(NCORES)))
    # supply_limit (*3) and fp16->fp32 upcast folded into the gather
    outs = [(r["y"].reshape(H_FULL, 3, WS).astype(np.float32) * 3.0)
            .transpose(0, 2, 1) for r in res.results]
    return np.ascontiguousarray(np.concatenate(outs, axis=1))
